# revision 40
# baseline (speedup 1.0000x reference)
"""BSplineSynapse Trainium2 kernel (8-core tensor-parallel over out_features).

Math: reference computes, with t = clip(|x|, 0, 1), s = 1 - t:
    w(t) = cp0*s^3 + 3*cp1*s^2*t + 3*cp2*s*t^2 + cp3*t^3   (per (o, i))
    out[b, o] = sum_i w[o, i](t[b, i]) * x[b, i]

Fast path (0 <= x <= 1, so t == x): w(x)*x is a quartic in x with zero
constant term. Rewritten in the CENTERED basis u = x - 1/2:
    out[b, o] = bias[o] + sum_j (u^j) @ E_j^T      (j = 1..4)
where E_j = recentred monomial weights and bias[o] = sum_i E_0[o, i].
The centered basis keeps both the moving tensors (|u^j| <= 2^-j) and the
weights small, so bf16 rounding stays ~3e-3 of max|out| (the naive
monomial basis at 0 gives ~1.5e-2 from cancellation amplification).

Per core (tensor-parallel over out_features, 128 rows each):
  - PE: 48 bf16 half-width matmuls (u,u2,u3 bases x 8 K-chunks x 2 batch
    halves, 256 moving cols) + 8 fp8e4 DoubleRow matmuls for the u4 term
    (two K-chunks each at 0.5 cyc/row; u4's small magnitude keeps the
    unscaled fp8 quantization inside the error budget - measured 6.9e-3).
    bias[o] is added for free during the PSUM drains (DVE
    tensor_scalar_add with a host-staged f32 per-partition column).
  - Inputs split over two parallel DMA queues (SP HWDGE + Pool SWDGE);
    u's first half arrives in octant slices so the PE starts ~1us in.
  - Elementwise: ACT does u2 = Square(u) (q0/q1), u4 = Square(u2); DVE
    does u2 q2/q3 (u*u) and u3 = u*u2, all quarter-granular to pipeline
    with the PE waves.
  - PE is kept continuously busy (a few warmup matmuls before the first
    octant lands) because the p-state ramp resets on idle gaps.
  - PSUM is split into two banks by batch half: bank A's matmuls finish
    first so its DVE drain + SP output DMA overlap the PE tail; bank B
    drains on ACT and goes out on the ACT HWDGE queue.

General path (any x): t = clip(|x|,0,1), u = t - 1/2 computed on host,
basis {x, u*x, u^2*x, u^3*x} with recentred cubic weights F_j; no bias.

Host staging only permutes/casts inputs and linearly recombines the cp
matrices (exact f64 math); all GEMM work runs on device.
"""

import sys

if "/opt/trn_rl_repo" not in sys.path:
    sys.path.insert(0, "/opt/trn_rl_repo")

from math import comb

import numpy as np
import ml_dtypes

import concourse.bacc as bacc
import concourse.mybir as mybir
from concourse.mybir import ActivationFunctionType as AF
from concourse.tile import TileContext
from concourse.bass_utils import run_bass_kernel_spmd

B = 512           # batch
I = 1024          # in_features
O = 1024          # out_features
NCORES = 8
OS = O // NCORES  # out_features per core = 128
CH = I // 128     # i-chunks of 128 = 8
W = CH * B        # staged x-side columns = 4096
WC = CH * OS      # staged weight columns = 1024

F32 = mybir.dt.float32
BF16 = mybir.dt.bfloat16
FP8 = mybir.dt.float8e4
BF = ml_dtypes.bfloat16
F8 = ml_dtypes.float8_e4m3fn

N_WARM = 5        # 128-row warmup matmuls before the first real matmul

_programs = {}


def _build_fast():
    nc = bacc.Bacc("TRN2", target_bir_lowering=False, debug=False)
    ud = nc.dram_tensor("u", [128, W], BF16, kind="ExternalInput")
    ed = [
        nc.dram_tensor(f"e{k}", [128, WC], BF16, kind="ExternalInput")
        for k in range(1, 4)
    ]
    ed.append(nc.dram_tensor("e4", [128, WC], FP8, kind="ExternalInput"))
    biasd = nc.dram_tensor("bias", [OS, 1], F32, kind="ExternalInput")
    outd = [
        nc.dram_tensor(f"out{h}", [OS, B // 2], F32, kind="ExternalOutput")
        for h in range(2)
    ]


    with TileContext(nc) as tc:
        with (
            tc.tile_pool(name="p", bufs=1) as pool,
            tc.tile_pool(name="ps", bufs=1, space="PSUM") as pp,
        ):
            u = pool.tile([128, W], BF16, tag="u", name="u")
            u2 = pool.tile([128, W], BF16, tag="u2", name="u2")
            u3 = pool.tile([128, W], BF16, tag="u3", name="u3")
            u4 = pool.tile([128, CH, 512], FP8, tag="u4", name="u4")
            e1a = pool.tile([128, 512], BF16, tag="e1a", name="e1a")
            e1b = pool.tile([128, 512], BF16, tag="e1b", name="e1b")
            e2 = pool.tile([128, WC], BF16, tag="e2", name="e2")
            e3 = pool.tile([128, WC], BF16, tag="e3", name="e3")
            e4 = pool.tile([128, CH, 128], FP8, tag="e4", name="e4")
            biasc = pool.tile([OS, 1], F32, tag="biasc", name="biasc")
            wsc = pool.tile([128, 128], BF16, tag="wsc", name="wsc")
            scr = pool.tile([1, 1], BF16, tag="scr", name="scr")
            osb = [
                pool.tile([128, B // 2], F32, tag=f"osb{h}", name=f"osb{h}")
                for h in range(2)
            ]
            ps_w = pp.tile([128, 128], F32, name="ps_w")
            psa = pp.tile([128, B // 2], F32, name="psa")
            psb = pp.tile([128, B // 2], F32, name="psb")

            def oct_(t, j):       # octant view: i-chunk j, 512 batch cols
                return t[:, j * 512:(j + 1) * 512]

            def q_(t, q):         # quarter view: i-chunks 2q..2q+1
                return t[:, q * 1024:(q + 1) * 1024]

            # ---- DMA queue programs (parallel queues) ----
            # Each DMA has a ~500ns queue-slot floor, so only the first two
            # octants go individually (early PE start); later pairs merge.
            # SP: u oct0, oct1, oct2+3, E2, E3; out half A at the end
            nc.sync.dma_start(out=oct_(u, 0), in_=ud.ap()[:, 0:512])
            nc.sync.dma_start(out=oct_(u, 1), in_=ud.ap()[:, 512:1024])
            nc.sync.dma_start(out=oct_(u, 2), in_=ud.ap()[:, 1024:1536])
            nc.sync.dma_start(out=oct_(u, 3), in_=ud.ap()[:, 1536:2048])
            nc.sync.dma_start(out=e2[:], in_=ed[1].ap())
            nc.sync.dma_start(out=e3[:], in_=ed[2].ap())
            # Pool (SWDGE): E1a, u oct4+5, oct6+7, E1b, E4, bias. SWDGE
            # completion sems are ~1us slower than HWDGE, so nothing
            # latency-critical goes last here and outputs avoid this queue.
            nc.gpsimd.dma_start(out=e1a[:], in_=ed[0].ap()[:, 0:512])
            nc.gpsimd.dma_start(out=u[:, 2048:3072], in_=ud.ap()[:, 2048:3072])
            nc.gpsimd.dma_start(out=u[:, 3072:4096], in_=ud.ap()[:, 3072:4096])
            nc.gpsimd.dma_start(out=e1b[:], in_=ed[0].ap()[:, 512:1024])
            nc.gpsimd.dma_start(out=biasc[:], in_=biasd.ap())
            nc.gpsimd.dma_start(out=e4[:], in_=ed[3].ap())

            # ---- elementwise producers ----
            # (emission order: every reader AFTER its writer in trace order;
            # per-engine execution order is the per-engine subsequence)
            nc.vector.memset(wsc[:], 1.0)
            # ACT: tiny dummy Square first so the activation table (1283ns)
            # loads before real operands arrive (output to scratch so the
            # PE warmup's wsc reads don't serialize behind it)
            nc.scalar.activation(scr[0:1, 0:1], wsc[0:1, 0:1], AF.Square)
            # ACT: u2 q0/q1 = Square(u); DVE: u2 q2/q3 = u*u
            nc.scalar.activation(q_(u2, 0), q_(u, 0), AF.Square)
            nc.scalar.activation(q_(u2, 1), q_(u, 1), AF.Square)
            nc.vector.tensor_mul(q_(u2, 2), q_(u, 2), q_(u, 2))
            nc.vector.tensor_mul(q_(u2, 3), q_(u, 3), q_(u, 3))
            for q in range(3):
                nc.vector.tensor_mul(q_(u3, q), q_(u, q), q_(u2, q))
            nc.gpsimd.tensor_mul(q_(u3, 3), q_(u, 3), q_(u2, 3))
            for q in range(3):
                nc.scalar.activation(u4[:, 2 * q:2 * q + 2, :], q_(u2, q), AF.Square)
            nc.gpsimd.tensor_mul(u4[:, 6:8, :], q_(u2, 3), q_(u2, 3))

            # ---- PE program: warmup then gap-free accumulation waves ----
            # PSUM is split by batch half (psa: b 0-255, psb: b 256-511);
            # bank B finishes first so its drain + DMA hide under the
            # final bank-A matmuls.
            for i in range(N_WARM):
                nc.tensor.matmul(
                    ps_w[:], lhsT=wsc[:], rhs=wsc[:],
                    start=(i == 0), stop=(i == N_WARM - 1),
                )
            na = [0]
            nb = [0]
            NA = 28  # 24 bf16 + 4 fp8-DoubleRow matmuls per bank

            def half_mm(ps, cnt, lhsT, rhs):
                nc.tensor.matmul(
                    ps[:], lhsT=lhsT, rhs=rhs,
                    start=(cnt[0] == 0), stop=(cnt[0] == NA - 1),
                )
                cnt[0] += 1

            def wave_oct(et, echunk, g, j, half=None):
                lhsT = et[:, echunk * 128:(echunk + 1) * 128]
                c0 = j * 512
                if half in (None, 0):
                    half_mm(psa, na, lhsT, g[:, c0:c0 + 256])
                if half in (None, 1):
                    half_mm(psb, nb, lhsT, g[:, c0 + 256:c0 + 512])

            def wave_q(et, g, q, half=None):
                for c in (2 * q, 2 * q + 1):
                    wave_oct(et, c, g, c, half)

            def wave_dr(cp, half=None):
                lhsT = e4[:, 2 * cp:2 * cp + 2, :]
                if half in (None, 0):
                    nc.tensor.matmul(
                        psa[:], lhsT=lhsT, rhs=u4[:, 2 * cp:2 * cp + 2, 0:256],
                        start=(na[0] == 0), stop=(na[0] == NA - 1),
                        perf_mode=mybir.MatmulPerfMode.DoubleRow,
                    )
                    na[0] += 1
                if half in (None, 1):
                    nc.tensor.matmul(
                        psb[:], lhsT=lhsT, rhs=u4[:, 2 * cp:2 * cp + 2, 256:512],
                        start=(nb[0] == 0), stop=(nb[0] == NA - 1),
                        perf_mode=mybir.MatmulPerfMode.DoubleRow,
                    )
                    nb[0] += 1

            # octant waves ordered by DMA arrival
            wave_oct(e1a, 0, u, 0)
            wave_oct(e1a, 1, u, 1)
            wave_oct(e1a, 2, u, 2)
            wave_oct(e1a, 3, u, 3)
            wave_oct(e1b, 0, u, 4)
            wave_oct(e1b, 1, u, 5)
            wave_oct(e1b, 2, u, 6)
            wave_q(e2, u2, 0)
            wave_oct(e1b, 3, u, 7)
            wave_q(e2, u2, 1)
            wave_q(e2, u2, 2)
            wave_q(e2, u2, 3)
            wave_q(e3, u3, 0)
            wave_dr(0)
            wave_q(e3, u3, 1)
            wave_dr(1)
            wave_q(e3, u3, 2)
            # tail: finish ALL of bank B first, then bank A. Bank B's ACT
            # drain + ACT-queue DMA (slower completion path) hide under the
            # remaining A matmuls; bank A's final drain goes out via the
            # faster SP completion path.
            wave_q(e3, u3, 3, half=1)
            wave_dr(2, half=1)
            wave_dr(3, half=1)
            # drain B emitted here: runs as soon as psb's stop fires;
            # bias[o] is added during the drain (f32, per-partition scalar)
            nc.vector.tensor_scalar_add(osb[1][:], psb[:], biasc[:])
            nc.scalar.dma_start(out=outd[1].ap(), in_=osb[1][:])
            wave_q(e3, u3, 3, half=0)
            wave_dr(2, half=0)
            wave_dr(3, half=0)
            assert na[0] == NA and nb[0] == NA

            nc.vector.tensor_scalar_add(osb[0][:], psa[:], biasc[:])
            nc.sync.dma_start(out=outd[0].ap(), in_=osb[0][:])

    nc.compile()
    return nc


def _build_general():
    nc = bacc.Bacc("TRN2", target_bir_lowering=False, debug=False)
    xd = nc.dram_tensor("x", [128, W], BF16, kind="ExternalInput")
    ud = nc.dram_tensor("u", [128, W], BF16, kind="ExternalInput")
    fd = [
        nc.dram_tensor(f"f{k}", [128, WC], BF16, kind="ExternalInput")
        for k in range(4)
    ]
    outd = [
        nc.dram_tensor(f"out{h}", [OS, B // 2], F32, kind="ExternalOutput")
        for h in range(2)
    ]

    with TileContext(nc) as tc:
        with (
            tc.tile_pool(name="p", bufs=1) as pool,
            tc.tile_pool(name="ps", bufs=1, space="PSUM") as pp,
        ):
            x = pool.tile([128, W], BF16, tag="x", name="x")
            u = pool.tile([128, W], BF16, tag="u", name="u")
            u2 = pool.tile([128, W], BF16, tag="u2", name="u2")
            m1 = pool.tile([128, W], BF16, tag="m1", name="m1")
            m2 = pool.tile([128, W], BF16, tag="m2", name="m2")
            m3 = pool.tile([128, W], BF16, tag="m3", name="m3")
            fs = [
                pool.tile([128, WC], BF16, tag=f"f{k}", name=f"f{k}")
                for k in range(4)
            ]
            wsc = pool.tile([128, 128], BF16, tag="wsc", name="wsc")
            osb = [
                pool.tile([128, B // 2], F32, tag=f"osb{h}", name=f"osb{h}")
                for h in range(2)
            ]
            ps_w = pp.tile([128, 128], F32, name="ps_w")
            psum = pp.tile([128, B], F32, name="psum")

            def oct_(t, j):
                return t[:, j * 512:(j + 1) * 512]

            def q_(t, q):
                return t[:, q * 1024:(q + 1) * 1024]

            # SP: x halves, f1, f3; Pool: f0, u halves, f2
            nc.sync.dma_start(out=x[:, 0:2048], in_=xd.ap()[:, 0:2048])
            nc.sync.dma_start(out=x[:, 2048:W], in_=xd.ap()[:, 2048:W])
            nc.sync.dma_start(out=fs[1][:], in_=fd[1].ap())
            nc.sync.dma_start(out=fs[3][:], in_=fd[3].ap())
            nc.gpsimd.dma_start(out=fs[0][:], in_=fd[0].ap())
            nc.gpsimd.dma_start(out=u[:, 0:2048], in_=ud.ap()[:, 0:2048])
            nc.gpsimd.dma_start(out=u[:, 2048:W], in_=ud.ap()[:, 2048:W])
            nc.gpsimd.dma_start(out=fs[2][:], in_=fd[2].ap())

            nc.vector.memset(wsc[:], 1.0)
            # DVE: m1 = u*x; m3 = u2*m1. ACT: u2 = Square(u); m2 = u2*x on DVE
            for q in range(4):
                nc.vector.tensor_mul(q_(m1, q), q_(u, q), q_(x, q))
            for q in range(4):
                nc.scalar.activation(q_(u2, q), q_(u, q), AF.Square)
            for q in range(4):
                nc.vector.tensor_mul(q_(m2, q), q_(u2, q), q_(x, q))
                nc.vector.tensor_mul(q_(m3, q), q_(u2, q), q_(m1, q))

            for i in range(N_WARM):
                nc.tensor.matmul(
                    ps_w[:], lhsT=wsc[:], rhs=wsc[:],
                    start=(i == 0), stop=(i == N_WARM - 1),
                )
            mm_n = [0]
            N_MM = 32

            def wave_oct(et, echunk, g, j):
                nc.tensor.matmul(
                    psum[:], lhsT=et[:, echunk * 128:(echunk + 1) * 128],
                    rhs=oct_(g, j),
                    start=(mm_n[0] == 0), stop=(mm_n[0] == N_MM - 1),
                )
                mm_n[0] += 1

            def wave_q(et, g, q):
                for c in (2 * q, 2 * q + 1):
                    wave_oct(et, c, g, c)

            for j in range(8):
                wave_oct(fs[0], j, x, j)
            for q in range(4):
                wave_q(fs[1], m1, q)
            for q in range(4):
                wave_q(fs[2], m2, q)
                wave_q(fs[3], m3, q)
            assert mm_n[0] == N_MM

            nc.vector.tensor_copy(osb[0][:], psum[:, 0:B // 2])
            nc.scalar.copy(osb[1][:], psum[:, B // 2:B])
            nc.sync.dma_start(out=outd[0].ap(), in_=osb[0][:])
            nc.gpsimd.dma_start(out=outd[1].ap(), in_=osb[1][:])

    nc.compile()
    return nc


def _get_program(fast: bool):
    if fast not in _programs:
        _programs[fast] = _build_fast() if fast else _build_general()
    return _programs[fast]


def _stage_xside(a):
    # [p, j*512 + b] = a[b, j*128 + p]  (f32/f64 in, bf16 out)
    st = a.T.reshape(CH, 128, B).transpose(1, 0, 2).reshape(128, W)
    return np.ascontiguousarray(st.astype(BF))


def _stage_w(wmat, core):
    # [p, c*128 + o] = wmat[o + 128*core, c*128 + p]
    sl = wmat[core * OS:(core + 1) * OS].T  # (1024, 128) [i, o]
    return np.ascontiguousarray(
        sl.reshape(CH, 128, OS).transpose(1, 0, 2).reshape(128, WC).astype(BF)
    )


def _weights(inputs, fast):
    cps = [np.asarray(inputs[f"cp{k}"], dtype=np.float64) for k in range(4)]
    # monomial (in t) coeffs of the cubic w(t)
    g = [
        cps[0],
        -3 * cps[0] + 3 * cps[1],
        3 * cps[0] - 6 * cps[1] + 3 * cps[2],
        -cps[0] + 3 * cps[1] - 3 * cps[2] + cps[3],
    ]
    if fast:
        # quartic p(x) = w(x)*x coeffs c_k (k=1..4 on x^k), recentred at 1/2
        c = [g[0], g[1], g[2], g[3]]  # c_{k+1} = g_k since p = w*x
        E = [
            sum(c[k - 1] * comb(k, j) * 0.5 ** (k - j) for k in range(max(j, 1), 5))
            for j in range(5)
        ]
        bias = E[0].sum(axis=1)  # (O,)
        return E[1:], bias
    else:
        # cubic w(t) recentred at 1/2: F_j, basis {x, ux, u^2 x, u^3 x}
        F = [
            sum(g[k] * comb(k, j) * 0.5 ** (k - j) for k in range(j, 4))
            for j in range(4)
        ]
        return F, None


def make_in_maps(inputs):
    x = np.asarray(inputs["x"], dtype=np.float64)
    fast = bool(x.min() >= 0.0) and bool(x.max() <= 1.0)
    if fast:
        E, bias = _weights(inputs, True)
        u_st = _stage_xside(x - 0.5)
        bias_f32 = bias.astype(np.float32)
        in_maps = []
        for c in range(NCORES):
            m = {"u": u_st}
            for k in range(3):
                m[f"e{k + 1}"] = _stage_w(E[k], c)
            m["e4"] = np.ascontiguousarray(
                _stage_w(E[3], c).astype(np.float32).astype(F8)
            )
            m["bias"] = np.ascontiguousarray(
                bias_f32[c * OS:(c + 1) * OS].reshape(OS, 1)
            )
            in_maps.append(m)
        return in_maps
    else:
        F, _ = _weights(inputs, False)
        t = np.clip(np.abs(x), 0.0, 1.0)
        x_st = _stage_xside(x)
        u_st = _stage_xside(t - 0.5)
        in_maps = []
        for c in range(NCORES):
            m = {"x": x_st, "u": u_st}
            for k in range(4):
                m[f"f{k}"] = _stage_w(F[k], c)
            in_maps.append(m)
        return in_maps


def kernel(**inputs) -> np.ndarray:
    x = np.asarray(inputs["x"], dtype=np.float32)
    fast = bool(x.min() >= 0.0) and bool(x.max() <= 1.0)
    nc = _get_program(fast)
    in_maps = make_in_maps(inputs)
    res = run_bass_kernel_spmd(nc, in_maps, core_ids=list(range(NCORES)))
    out = np.empty((B, O), dtype=np.float32)
    for c in range(NCORES):
        sl = slice(c * OS, (c + 1) * OS)
        out[: B // 2, sl] = res.results[c]["out0"].T
        out[B // 2:, sl] = res.results[c]["out1"].T
    return out
